# revision 4
# baseline (speedup 1.0000x reference)
"""Trainium2 Bass kernel for GQA causal self-attention with ALiBi.

Model (reference):
  B=2, L=2048, H=1024, n_head=16, n_kv=4 (GQA groups of 4 q-heads), D=64
  q = x @ Wq.T ; kv = x @ Wkv.T ; scores = SCALE*q@k.T + alibi ; causal softmax
  out = (softmax @ v) head-concat @ Wo.T

Sharding: 8 cores = 2 batches x 4 kv-groups (data + head/tensor parallel).
Each core computes its batch's projections for its kv-group (4 q-heads,
1 k/v head), full causal flash-attention for those heads, and a partial
out-projection (its 256 columns of Wo). Host sums the 4 partials per batch.

Math notes:
 - SCALE folded into Wq on host.
 - ALiBi + causal: softmax_j(s + slope*(j-i)) == softmax_j(s + slope*j + const_i).
   The per-j term slope*j is applied as the (exact, fp32) per-partition bias of
   the ScalarE exp; the per-i stability shift (-slope*i - C) rides a rank-1
   matmul augmentation row (bf16 rounding of it cancels exactly in softmax).
 - Scores are computed transposed, sT[j, i], so no on-chip transposes of the
   softmax matrix are needed; v is transposed once via DMA-transpose.
 - Softmax denominator comes free as an extra ones-column of the v operand.
"""

import sys
import types

import numpy as np
import ml_dtypes

import concourse.bass as bass
import concourse.tile as tile
import concourse.mybir as mybir
from concourse import bacc
from concourse.bass_utils import run_bass_kernel_spmd

B, L, H = 2, 2048, 1024
N_HEAD, N_KV, D = 16, 4, 64
QPK = N_HEAD // N_KV  # 4 q-heads per core
SCALE = D ** -0.5
C_STAB = 10.0
N_CORES = 8
NKT = H // 128  # 8 contraction tiles
NJT = L // 128  # 16 key tiles
BIG = 1024      # i-chunk width (2 PSUM banks)
NCH = L // BIG  # 2 i-chunks

BF16 = mybir.dt.bfloat16
F32 = mybir.dt.float32
nbf16 = ml_dtypes.bfloat16


def _ensure_ntff_hook():
    """Shim antenv.axon_hooks (absent in this image) so trace=True works."""
    if "antenv.axon_hooks" in sys.modules:
        return
    try:
        from trn_agent_boot.trn_boot import _ntff_profile_via_ctypes
        hook = _ntff_profile_via_ctypes("/opt/axon/libaxon_pjrt.so")
    except Exception:
        hook = None
    mod = types.ModuleType("antenv.axon_hooks")
    mod.get_axon_ntff_profile_hook = lambda: hook
    sys.modules["antenv.axon_hooks"] = mod


def build_bass():
    nc = bacc.Bacc("TRN2", target_bir_lowering=False, debug=False,
                   num_devices=N_CORES)
    xt_d = nc.dram_tensor("xt", [H, L], BF16, kind="ExternalInput")
    wq_d = nc.dram_tensor("wq", [H, 2 * 128], BF16, kind="ExternalInput")
    wk_d = nc.dram_tensor("wk", [H, D], BF16, kind="ExternalInput")
    wv_d = nc.dram_tensor("wv", [H, D], BF16, kind="ExternalInput")
    wo_d = nc.dram_tensor("wo", [2 * 128, H], BF16, kind="ExternalInput")
    alibi_d = nc.dram_tensor("alibi", [128, QPK * NJT], F32, kind="ExternalInput")
    qaug_d = nc.dram_tensor("qaug", [QPK, L], BF16, kind="ExternalInput")
    mask_d = nc.dram_tensor("mask", [128, 128], BF16, kind="ExternalInput")
    yt_d = nc.dram_tensor("yt", [H, L], F32, kind="ExternalOutput")

    with tile.TileContext(nc) as tc:
        with (
            tc.tile_pool(name="consts", bufs=1) as consts,
            tc.tile_pool(name="pt_pool", bufs=3) as pt_pool,
            tc.tile_pool(name="norm_pool", bufs=2) as norm_pool,
            tc.tile_pool(name="y_pool", bufs=3) as y_pool,
            tc.tile_pool(name="psA", bufs=2, space="PSUM") as psA,
            tc.tile_pool(name="psS", bufs=2, space="PSUM") as psS,
            tc.tile_pool(name="psV", bufs=1, space="PSUM") as psV,
        ):
            # ---- persistent SBUF tensors ----
            xt = consts.tile([128, NKT, L], BF16)
            wq = consts.tile([128, NKT, 2 * 128], BF16)
            wk = consts.tile([128, NKT, D], BF16)
            wv = consts.tile([128, NKT, D], BF16)
            wo = consts.tile([128, 2, H], BF16)
            alibi = consts.tile([128, QPK * NJT], F32)
            mask = consts.tile([128, 128], BF16)
            qaug = consts.tile([D + 1, QPK, L], BF16)
            kaug = consts.tile([D + 1, L], BF16)
            vaug = consts.tile([128, NJT, D + 1], BF16)
            vtmp = consts.tile([D, L], BF16)
            attnT = consts.tile([128, 2, L], BF16)

            # ---- input DMAs ----
            for kt in range(NKT):
                nc.sync.dma_start(xt[:, kt, :], xt_d[128 * kt:128 * (kt + 1), :])
                nc.sync.dma_start(wq[:, kt, :], wq_d[128 * kt:128 * (kt + 1), :])
                nc.sync.dma_start(wk[:, kt, :], wk_d[128 * kt:128 * (kt + 1), :])
                nc.sync.dma_start(wv[:, kt, :], wv_d[128 * kt:128 * (kt + 1), :])
            nc.sync.dma_start(wo[:, 0, :], wo_d[0:128, :])
            nc.sync.dma_start(wo[:, 1, :], wo_d[128:256, :])
            nc.sync.dma_start(alibi[:], alibi_d[:])
            nc.sync.dma_start(mask[:], mask_d[:])
            for p in range(QPK):
                nc.sync.dma_start(qaug[D:D + 1, p, :], qaug_d[p:p + 1, :])
            nc.vector.memset(kaug[D:D + 1, :], 1.0)
            nc.vector.memset(vaug[:, :, D:D + 1], 1.0)

            # ---- K / V projections: kT/vT [64, L] ----
            for l in range(L // 512):
                sl = slice(512 * l, 512 * (l + 1))
                pk = psA.tile([128, 512], F32, tag="proj")
                for kt in range(NKT):
                    nc.tensor.matmul(pk[0:D, :], wk[:, kt, :], xt[:, kt, sl],
                                     start=(kt == 0), stop=(kt == NKT - 1))
                nc.vector.tensor_copy(kaug[0:D, sl], pk[0:D, :])
                pv_ = psA.tile([128, 512], F32, tag="proj")
                for kt in range(NKT):
                    nc.tensor.matmul(pv_[0:D, :], wv[:, kt, :], xt[:, kt, sl],
                                     start=(kt == 0), stop=(kt == NKT - 1))
                nc.vector.tensor_copy(vtmp[:, sl], pv_[0:D, :])
            # v natural layout via DMA transpose: vnat[:, jt, :] = vtmp[:, jt-block].T
            # (transpose dest must be 128B-aligned, so go via a packed tile)
            vnat = consts.tile([128, NJT, D], BF16)
            for jt in range(NJT):
                nc.sync.dma_start_transpose(vnat[:, jt, :],
                                            vtmp[:, 128 * jt:128 * (jt + 1)])
                nc.gpsimd.tensor_copy(vaug[:, jt, 0:D], vnat[:, jt, :])

            # ---- Q projection -> qaug rows 0..63 per head ----
            for m in range(2):
                for l in range(L // 512):
                    sl = slice(512 * l, 512 * (l + 1))
                    pq = psA.tile([128, 512], F32, tag="proj")
                    for kt in range(NKT):
                        nc.tensor.matmul(pq[:], wq[:, kt, 128 * m:128 * (m + 1)],
                                         xt[:, kt, sl],
                                         start=(kt == 0), stop=(kt == NKT - 1))
                    nc.vector.tensor_copy(qaug[0:D, 2 * m, sl], pq[0:D, :])
                    nc.vector.tensor_copy(qaug[0:D, 2 * m + 1, sl], pq[D:128, :])

            # ---- attention per head / per 1024-wide i-chunk ----
            for p in range(QPK):
                for k2 in range(NCH):
                    i0 = BIG * k2
                    pv = psV.tile([D + 1, BIG], F32)
                    last_jt = 8 * k2 + 7
                    for jt in range(last_jt + 1):
                        off = max(0, 128 * jt - i0)
                        pieces = ([(off, 512), (512, BIG)] if off < 512
                                  else [(off, BIG)])
                        st = psS.tile([128, BIG], F32)
                        for (a, b) in pieces:
                            nc.tensor.matmul(
                                st[:, a:b],
                                kaug[:, 128 * jt:128 * (jt + 1)],
                                qaug[:, p, i0 + a:i0 + b],
                                start=True, stop=True)
                        pt = pt_pool.tile([128, BIG], BF16)
                        nc.scalar.activation(
                            pt[:, off:BIG], st[:, off:BIG],
                            mybir.ActivationFunctionType.Exp,
                            bias=alibi[:, p * NJT + jt:p * NJT + jt + 1])
                        if 128 * jt >= i0:  # diagonal tile: causal mask
                            nc.gpsimd.tensor_mul(pt[:, off:off + 128],
                                                 pt[:, off:off + 128], mask[:])
                        for (a, b) in pieces:
                            bank_last = (last_jt if b == BIG
                                         else min(8 * k2 + 3, last_jt))
                            nc.tensor.matmul(
                                pv[:, a:b], vaug[:, jt, :], pt[:, a:b],
                                start=(jt == 0), stop=(jt == bank_last))
                    # normalization: attnT[head rows, chunk] = pv[0:64]/pv[64]
                    pvs = norm_pool.tile([D, BIG], F32, tag="pvs")
                    nc.vector.tensor_copy(pvs[:], pv[0:D, :])
                    den = norm_pool.tile([1, BIG], F32, tag="den")
                    nc.scalar.copy(den[:], pv[D:D + 1, :])
                    rec = norm_pool.tile([1, BIG], F32, tag="rec")
                    nc.vector.reciprocal_approx_fast(rec[:], den[:])
                    recb = norm_pool.tile([D, BIG], F32, tag="recb")
                    nc.gpsimd.partition_broadcast(recb[:], rec[:])
                    nc.gpsimd.tensor_mul(
                        attnT[64 * (p % 2):64 * (p % 2) + D, p // 2,
                              i0:i0 + BIG],
                        pvs[:], recb[:])

            # ---- partial out-projection: yT[o, i] ----
            for m in range(H // 128):
                for l in range(L // 512):
                    sl = slice(512 * l, 512 * (l + 1))
                    py = psA.tile([128, 512], F32, tag="proj")
                    for c2 in range(2):
                        nc.tensor.matmul(py[:], wo[:, c2, 128 * m:128 * (m + 1)],
                                         attnT[:, c2, sl],
                                         start=(c2 == 0), stop=(c2 == 1))
                    ys = y_pool.tile([128, 512], F32)
                    if (m * 4 + l) % 2 == 0:
                        nc.scalar.copy(ys[:], py[:])
                    else:
                        nc.vector.tensor_copy(ys[:], py[:])
                    nc.sync.dma_start(yt_d[128 * m:128 * (m + 1), sl], ys[:])

    nc.compile()
    return nc


_NC_CACHE = None


def _get_nc():
    global _NC_CACHE
    if _NC_CACHE is None:
        _NC_CACHE = build_bass()
    return _NC_CACHE


def make_in_maps(x, Wq, Wkv, Wo):
    x = np.asarray(x, np.float32)
    Wq = np.asarray(Wq, np.float32)
    Wkv = np.asarray(Wkv, np.float32)
    Wo = np.asarray(Wo, np.float32)

    slopes = 2.0 ** (-8.0 / N_HEAD * (np.arange(N_HEAD, dtype=np.float64) + 1.0))
    jpos = np.arange(128, dtype=np.float64)
    ipos = np.arange(L, dtype=np.float64)
    mask = np.where(jpos[:, None] <= jpos[None, :], 1.0, 0.0).astype(nbf16)

    in_maps = []
    for core in range(N_CORES):
        b, g = divmod(core, N_KV)
        heads = [N_KV * 0 + 4 * g + p for p in range(QPK)]  # 4g..4g+3
        xt = np.ascontiguousarray(x[b].T).astype(nbf16)
        wq = np.ascontiguousarray(
            (Wq[256 * g:256 * (g + 1), :] * SCALE).T).astype(nbf16)
        wk = np.ascontiguousarray(Wkv[128 * g:128 * g + D, :].T).astype(nbf16)
        wv = np.ascontiguousarray(Wkv[128 * g + D:128 * (g + 1), :].T).astype(nbf16)
        wo = np.ascontiguousarray(Wo[:, 256 * g:256 * (g + 1)].T).astype(nbf16)
        alibi = np.empty((128, QPK * NJT), np.float32)
        for p in range(QPK):
            s = slopes[heads[p]]
            for jt in range(NJT):
                alibi[:, p * NJT + jt] = (s * (128 * jt + jpos)).astype(np.float32)
        qaug = np.empty((QPK, L), nbf16)
        for p in range(QPK):
            s = slopes[heads[p]]
            qaug[p] = (-s * ipos - C_STAB).astype(nbf16)
        in_maps.append({
            "xt": xt, "wq": wq, "wk": wk, "wv": wv, "wo": wo,
            "alibi": alibi, "qaug": qaug, "mask": mask,
        })
    return in_maps


def kernel(x, Wq, Wkv, Wo, _trace=False):
    _ensure_ntff_hook()
    nc = _get_nc()
    in_maps = make_in_maps(x, Wq, Wkv, Wo)
    res = run_bass_kernel_spmd(nc, in_maps, core_ids=list(range(N_CORES)),
                               trace=_trace)
    outs = [r["yt"] for r in res.results]  # each [H, L] = partial y.T
    y = np.empty((B, L, H), np.float32)
    for b in range(B):
        acc = outs[N_KV * b]
        for g in range(1, N_KV):
            acc = acc + outs[N_KV * b + g]
        y[b] = acc.T
    if _trace:
        kernel._last_result = res
    return y


# revision 5
# speedup vs baseline: 1.4119x; 1.4119x over previous
"""Trainium2 Bass kernel for GQA causal self-attention with ALiBi.

Model (reference):
  B=2, L=2048, H=1024, n_head=16, n_kv=4 (GQA groups of 4 q-heads), D=64
  q = x @ Wq.T ; kv = x @ Wkv.T ; scores = SCALE*q@k.T + alibi ; causal softmax
  out = (softmax @ v) head-concat @ Wo.T

Sharding: 8 cores = 2 batches x 4 kv-groups (data + head/tensor parallel).
Each core computes its batch's projections for its kv-group (4 q-heads,
1 k/v head), full causal flash-attention for those heads, and a partial
out-projection (its 256 columns of Wo). Host sums the 4 partials per batch.

Math notes:
 - SCALE folded into Wq on host.
 - ALiBi + causal: softmax_j(s + slope*(j-i)) == softmax_j(s + slope*j + const_i).
   The per-j term slope*j is applied as the (exact, fp32) per-partition bias of
   the ScalarE exp; the per-i stability shift (-slope*i - C) rides a rank-1
   matmul augmentation row (bf16 rounding of it cancels exactly in softmax).
 - Scores are computed transposed, sT[j, i], so no on-chip transposes of the
   softmax matrix are needed; v is transposed once via DMA-transpose.
 - Softmax denominator comes free as an extra ones-column of the v operand.
"""

import sys
import types

import numpy as np
import ml_dtypes

import concourse.bass as bass
import concourse.tile as tile
import concourse.mybir as mybir
from concourse import bacc
from concourse.bass_utils import run_bass_kernel_spmd

B, L, H = 2, 2048, 1024
N_HEAD, N_KV, D = 16, 4, 64
QPK = N_HEAD // N_KV  # 4 q-heads per core
SCALE = D ** -0.5
C_STAB = 10.0
N_CORES = 8
NKT = H // 128  # 8 contraction tiles
NJT = L // 128  # 16 key tiles
BIG = 1024      # i-chunk width (2 PSUM banks)
NCH = L // BIG  # 2 i-chunks

BF16 = mybir.dt.bfloat16
F32 = mybir.dt.float32
nbf16 = ml_dtypes.bfloat16


def _ensure_ntff_hook():
    """Shim antenv.axon_hooks (absent in this image) so trace=True works."""
    if "antenv.axon_hooks" in sys.modules:
        return
    try:
        from trn_agent_boot.trn_boot import _ntff_profile_via_ctypes
        hook = _ntff_profile_via_ctypes("/opt/axon/libaxon_pjrt.so")
    except Exception:
        hook = None
    mod = types.ModuleType("antenv.axon_hooks")
    mod.get_axon_ntff_profile_hook = lambda: hook
    sys.modules["antenv.axon_hooks"] = mod


def build_bass():
    nc = bacc.Bacc("TRN2", target_bir_lowering=False, debug=False,
                   num_devices=N_CORES)
    xt_d = nc.dram_tensor("xt", [H, L], BF16, kind="ExternalInput")
    wq_d = nc.dram_tensor("wq", [H, 2 * 128], BF16, kind="ExternalInput")
    wk_d = nc.dram_tensor("wk", [H, D], BF16, kind="ExternalInput")
    wv_d = nc.dram_tensor("wv", [H, D], BF16, kind="ExternalInput")
    wo_d = nc.dram_tensor("wo", [2 * 128, H], BF16, kind="ExternalInput")
    alibi_d = nc.dram_tensor("alibi", [128, QPK * NJT], F32, kind="ExternalInput")
    qaug_d = nc.dram_tensor("qaug", [QPK, L], BF16, kind="ExternalInput")
    mask_d = nc.dram_tensor("mask", [128, 128], BF16, kind="ExternalInput")
    yt_d = nc.dram_tensor("yt", [H, L], F32, kind="ExternalOutput")

    with tile.TileContext(nc) as tc:
        with (
            tc.tile_pool(name="consts", bufs=1) as consts,
            tc.tile_pool(name="pt_pool", bufs=3) as pt_pool,
            tc.tile_pool(name="norm_pool", bufs=2) as norm_pool,
            tc.tile_pool(name="y_pool", bufs=3) as y_pool,
            tc.tile_pool(name="psA", bufs=2, space="PSUM") as psA,
            tc.tile_pool(name="psS", bufs=2, space="PSUM") as psS,
            tc.tile_pool(name="psV", bufs=1, space="PSUM") as psV,
        ):
            # ---- persistent SBUF tensors ----
            xt = consts.tile([128, NKT, L], BF16)
            wq = consts.tile([128, NKT, 2 * 128], BF16)
            wk = consts.tile([128, NKT, D], BF16)
            wv = consts.tile([128, NKT, D], BF16)
            wo = consts.tile([128, 2, H], BF16)
            alibi = consts.tile([128, QPK * NJT], F32)
            mask = consts.tile([128, 128], BF16)
            qaug = consts.tile([D + 1, QPK, L], BF16)
            kaug = consts.tile([D + 1, L], BF16)
            vaug = consts.tile([128, NJT, D + 1], BF16)
            vtmp = consts.tile([D, L], BF16)
            attnT = consts.tile([128, 2, L], BF16)

            # ---- input DMAs ----
            for kt in range(NKT):
                nc.sync.dma_start(wq[:, kt, :], wq_d[128 * kt:128 * (kt + 1), :])
                nc.sync.dma_start(wk[:, kt, :], wk_d[128 * kt:128 * (kt + 1), :])
                nc.sync.dma_start(wv[:, kt, :], wv_d[128 * kt:128 * (kt + 1), :])
            for l in range(L // 512):
                for kt in range(NKT):
                    nc.sync.dma_start(
                        xt[:, kt, 512 * l:512 * (l + 1)],
                        xt_d[128 * kt:128 * (kt + 1), 512 * l:512 * (l + 1)])
            nc.sync.dma_start(wo[:, 0, :], wo_d[0:128, :])
            nc.sync.dma_start(wo[:, 1, :], wo_d[128:256, :])
            nc.sync.dma_start(alibi[:], alibi_d[:])
            nc.sync.dma_start(mask[:], mask_d[:])
            for p in range(QPK):
                nc.sync.dma_start(qaug[D:D + 1, p, :], qaug_d[p:p + 1, :])
            nc.vector.memset(kaug[D:D + 1, :], 1.0)
            nc.vector.memset(vaug[:, :, D:D + 1], 1.0)

            # ---- K / V projections: kT/vT [64, L] ----
            for l in range(L // 512):
                sl = slice(512 * l, 512 * (l + 1))
                pk = psA.tile([128, 512], F32, tag="proj")
                for kt in range(NKT):
                    nc.tensor.matmul(pk[0:D, :], wk[:, kt, :], xt[:, kt, sl],
                                     start=(kt == 0), stop=(kt == NKT - 1))
                nc.vector.tensor_copy(kaug[0:D, sl], pk[0:D, :])
                pv_ = psA.tile([128, 512], F32, tag="proj")
                for kt in range(NKT):
                    nc.tensor.matmul(pv_[0:D, :], wv[:, kt, :], xt[:, kt, sl],
                                     start=(kt == 0), stop=(kt == NKT - 1))
                nc.vector.tensor_copy(vtmp[:, sl], pv_[0:D, :])
            # v natural layout via DMA transpose: vnat[:, jt, :] = vtmp[:, jt-block].T
            # (transpose dest must be 128B-aligned, so go via a packed tile)
            vnat = consts.tile([128, NJT, D], BF16)
            for jt in range(NJT):
                nc.sync.dma_start_transpose(vnat[:, jt, :],
                                            vtmp[:, 128 * jt:128 * (jt + 1)])
                nc.gpsimd.tensor_copy(vaug[:, jt, 0:D], vnat[:, jt, :])

            # ---- Q projection -> qaug rows 0..63 per head ----
            for m in range(2):
                for l in range(L // 512):
                    sl = slice(512 * l, 512 * (l + 1))
                    pq = psA.tile([128, 512], F32, tag="proj")
                    for kt in range(NKT):
                        nc.tensor.matmul(pq[:], wq[:, kt, 128 * m:128 * (m + 1)],
                                         xt[:, kt, sl],
                                         start=(kt == 0), stop=(kt == NKT - 1))
                    nc.vector.tensor_copy(qaug[0:D, 2 * m, sl], pq[0:D, :])
                    nc.vector.tensor_copy(qaug[0:D, 2 * m + 1, sl], pq[D:128, :])

            # ---- attention per head / per 1024-wide i-chunk ----
            for p in range(QPK):
                for k2 in range(NCH):
                    i0 = BIG * k2
                    pv = psV.tile([D + 1, BIG], F32)
                    last_jt = 8 * k2 + 7
                    for jt in range(last_jt + 1):
                        off = max(0, 128 * jt - i0)
                        pieces = ([(off, 512), (512, BIG)] if off < 512
                                  else [(off, BIG)])
                        st = psS.tile([128, BIG], F32)
                        for (a, b) in pieces:
                            nc.tensor.matmul(
                                st[:, a:b],
                                kaug[:, 128 * jt:128 * (jt + 1)],
                                qaug[:, p, i0 + a:i0 + b],
                                start=True, stop=True)
                        pt = pt_pool.tile([128, BIG], BF16)
                        nc.scalar.activation(
                            pt[:, off:BIG], st[:, off:BIG],
                            mybir.ActivationFunctionType.Exp,
                            bias=alibi[:, p * NJT + jt:p * NJT + jt + 1])
                        if 128 * jt >= i0:  # diagonal tile: causal mask
                            nc.vector.tensor_mul(pt[:, off:off + 128],
                                                 pt[:, off:off + 128], mask[:])
                        for (a, b) in pieces:
                            bank_last = (last_jt if b == BIG
                                         else min(8 * k2 + 3, last_jt))
                            nc.tensor.matmul(
                                pv[:, a:b], vaug[:, jt, :], pt[:, a:b],
                                start=(jt == 0), stop=(jt == bank_last))
                    # normalization: attnT[head rows, chunk] = pv[0:64]/pv[64]
                    pvs = norm_pool.tile([D, BIG], F32, tag="pvs")
                    nc.vector.tensor_copy(pvs[:], pv[0:D, :])
                    den = norm_pool.tile([1, BIG], F32, tag="den")
                    nc.scalar.copy(den[:], pv[D:D + 1, :])
                    rec = norm_pool.tile([1, BIG], F32, tag="rec")
                    nc.vector.reciprocal_approx_fast(rec[:], den[:])
                    recb = norm_pool.tile([D, BIG], F32, tag="recb")
                    nc.gpsimd.partition_broadcast(recb[:], rec[:])
                    nc.vector.tensor_mul(
                        attnT[64 * (p % 2):64 * (p % 2) + D, p // 2,
                              i0:i0 + BIG],
                        pvs[:], recb[:])

            # ---- partial out-projection: yT[o, i] ----
            for m in range(H // 128):
                for l in range(L // 512):
                    sl = slice(512 * l, 512 * (l + 1))
                    py = psA.tile([128, 512], F32, tag="proj")
                    for c2 in range(2):
                        nc.tensor.matmul(py[:], wo[:, c2, 128 * m:128 * (m + 1)],
                                         attnT[:, c2, sl],
                                         start=(c2 == 0), stop=(c2 == 1))
                    ys = y_pool.tile([128, 512], F32)
                    if (m * 4 + l) % 2 == 0:
                        nc.scalar.copy(ys[:], py[:])
                    else:
                        nc.vector.tensor_copy(ys[:], py[:])
                    nc.sync.dma_start(yt_d[128 * m:128 * (m + 1), sl], ys[:])

    nc.compile()
    return nc


_NC_CACHE = None


def _get_nc():
    global _NC_CACHE
    if _NC_CACHE is None:
        _NC_CACHE = build_bass()
    return _NC_CACHE


def make_in_maps(x, Wq, Wkv, Wo):
    x = np.asarray(x, np.float32)
    Wq = np.asarray(Wq, np.float32)
    Wkv = np.asarray(Wkv, np.float32)
    Wo = np.asarray(Wo, np.float32)

    slopes = 2.0 ** (-8.0 / N_HEAD * (np.arange(N_HEAD, dtype=np.float64) + 1.0))
    jpos = np.arange(128, dtype=np.float64)
    ipos = np.arange(L, dtype=np.float64)
    mask = np.where(jpos[:, None] <= jpos[None, :], 1.0, 0.0).astype(nbf16)

    in_maps = []
    for core in range(N_CORES):
        b, g = divmod(core, N_KV)
        heads = [N_KV * 0 + 4 * g + p for p in range(QPK)]  # 4g..4g+3
        xt = np.ascontiguousarray(x[b].T).astype(nbf16)
        wq = np.ascontiguousarray(
            (Wq[256 * g:256 * (g + 1), :] * SCALE).T).astype(nbf16)
        wk = np.ascontiguousarray(Wkv[128 * g:128 * g + D, :].T).astype(nbf16)
        wv = np.ascontiguousarray(Wkv[128 * g + D:128 * (g + 1), :].T).astype(nbf16)
        wo = np.ascontiguousarray(Wo[:, 256 * g:256 * (g + 1)].T).astype(nbf16)
        alibi = np.empty((128, QPK * NJT), np.float32)
        for p in range(QPK):
            s = slopes[heads[p]]
            for jt in range(NJT):
                alibi[:, p * NJT + jt] = (s * (128 * jt + jpos)).astype(np.float32)
        qaug = np.empty((QPK, L), nbf16)
        for p in range(QPK):
            s = slopes[heads[p]]
            qaug[p] = (-s * ipos - C_STAB).astype(nbf16)
        in_maps.append({
            "xt": xt, "wq": wq, "wk": wk, "wv": wv, "wo": wo,
            "alibi": alibi, "qaug": qaug, "mask": mask,
        })
    return in_maps


def kernel(x, Wq, Wkv, Wo, _trace=False):
    _ensure_ntff_hook()
    nc = _get_nc()
    in_maps = make_in_maps(x, Wq, Wkv, Wo)
    res = run_bass_kernel_spmd(nc, in_maps, core_ids=list(range(N_CORES)),
                               trace=_trace)
    outs = [r["yt"] for r in res.results]  # each [H, L] = partial y.T
    y = np.empty((B, L, H), np.float32)
    for b in range(B):
        acc = outs[N_KV * b]
        for g in range(1, N_KV):
            acc = acc + outs[N_KV * b + g]
        y[b] = acc.T
    if _trace:
        kernel._last_result = res
    return y


# revision 6
# speedup vs baseline: 1.4294x; 1.0124x over previous
"""Trainium2 Bass kernel for GQA causal self-attention with ALiBi.

Model (reference):
  B=2, L=2048, H=1024, n_head=16, n_kv=4 (GQA groups of 4 q-heads), D=64
  q = x @ Wq.T ; kv = x @ Wkv.T ; scores = SCALE*q@k.T + alibi ; causal softmax
  out = (softmax @ v) head-concat @ Wo.T

Sharding: 8 cores = 2 batches x 4 kv-groups (data + head/tensor parallel).
Each core computes its batch's projections for its kv-group (4 q-heads,
1 k/v head), full causal flash-attention for those heads, and a partial
out-projection (its 256 columns of Wo). Host sums the 4 partials per batch.

Math notes:
 - SCALE folded into Wq on host.
 - ALiBi + causal: softmax_j(s + slope*(j-i)) == softmax_j(s + slope*j + const_i).
   The per-j term slope*j is applied as the (exact, fp32) per-partition bias of
   the ScalarE exp; the per-i stability shift (-slope*i - C) rides a rank-1
   matmul augmentation row (bf16 rounding of it cancels exactly in softmax).
 - Scores are computed transposed, sT[j, i], so no on-chip transposes of the
   softmax matrix are needed; v is transposed once via DMA-transpose.
 - Softmax denominator comes free as an extra ones-column of the v operand.
"""

import sys
import types

import numpy as np
import ml_dtypes

import concourse.bass as bass
import concourse.tile as tile
import concourse.mybir as mybir
from concourse import bacc
from concourse.bass_utils import run_bass_kernel_spmd

B, L, H = 2, 2048, 1024
N_HEAD, N_KV, D = 16, 4, 64
QPK = N_HEAD // N_KV  # 4 q-heads per core
SCALE = D ** -0.5
C_STAB = 10.0
N_CORES = 8
NKT = H // 128  # 8 contraction tiles
NJT = L // 128  # 16 key tiles
BIG = 1024      # i-chunk width (2 PSUM banks)
NCH = L // BIG  # 2 i-chunks

BF16 = mybir.dt.bfloat16
F32 = mybir.dt.float32
nbf16 = ml_dtypes.bfloat16


def _ensure_ntff_hook():
    """Shim antenv.axon_hooks (absent in this image) so trace=True works."""
    if "antenv.axon_hooks" in sys.modules:
        return
    try:
        from trn_agent_boot.trn_boot import _ntff_profile_via_ctypes
        hook = _ntff_profile_via_ctypes("/opt/axon/libaxon_pjrt.so")
    except Exception:
        hook = None
    mod = types.ModuleType("antenv.axon_hooks")
    mod.get_axon_ntff_profile_hook = lambda: hook
    sys.modules["antenv.axon_hooks"] = mod


def build_bass():
    nc = bacc.Bacc("TRN2", target_bir_lowering=False, debug=False,
                   num_devices=N_CORES)
    xt_d = nc.dram_tensor("xt", [H, L], BF16, kind="ExternalInput")
    wq_d = nc.dram_tensor("wq", [H, 2 * 128], BF16, kind="ExternalInput")
    wkv_d = nc.dram_tensor("wkv", [H, 128], BF16, kind="ExternalInput")
    wo_d = nc.dram_tensor("wo", [2 * 128, H], BF16, kind="ExternalInput")
    alibi_d = nc.dram_tensor("alibi", [128, QPK * NJT], F32, kind="ExternalInput")
    qaug_d = nc.dram_tensor("qaug", [QPK, L], BF16, kind="ExternalInput")
    mask_d = nc.dram_tensor("mask", [128, 128], BF16, kind="ExternalInput")
    yt_d = nc.dram_tensor("yt", [H, L], F32, kind="ExternalOutput")

    with tile.TileContext(nc) as tc:
        with (
            tc.tile_pool(name="consts", bufs=1) as consts,
            tc.tile_pool(name="pt_pool", bufs=3) as pt_pool,
            tc.tile_pool(name="norm_pool", bufs=2) as norm_pool,
            tc.tile_pool(name="y_pool", bufs=3) as y_pool,
            tc.tile_pool(name="ps", bufs=1, space="PSUM") as ps,
        ):
            # ---- persistent SBUF tensors ----
            xt = consts.tile([128, NKT, L], BF16)
            wq = consts.tile([128, NKT, 2 * 128], BF16)
            wkv = consts.tile([128, NKT, 128], BF16)
            wo = consts.tile([128, 2, H], BF16)
            alibi = consts.tile([128, QPK * NJT], F32)
            mask = consts.tile([128, 128], BF16)
            qaug = consts.tile([D + 1, QPK, L], BF16)
            kaug = consts.tile([D + 1, L], BF16)
            vaug = consts.tile([128, NJT, D + 1], BF16)
            vtmp = consts.tile([D, L], BF16)
            attnT = consts.tile([128, 2, L], BF16)

            # ---- input DMAs ----
            for kt in range(NKT):
                nc.sync.dma_start(wq[:, kt, :], wq_d[128 * kt:128 * (kt + 1), :])
                nc.sync.dma_start(wkv[:, kt, :], wkv_d[128 * kt:128 * (kt + 1), :])
            for l in range(L // 512):
                for kt in range(NKT):
                    nc.sync.dma_start(
                        xt[:, kt, 512 * l:512 * (l + 1)],
                        xt_d[128 * kt:128 * (kt + 1), 512 * l:512 * (l + 1)])
            nc.sync.dma_start(wo[:, 0, :], wo_d[0:128, :])
            nc.sync.dma_start(wo[:, 1, :], wo_d[128:256, :])
            nc.sync.dma_start(alibi[:], alibi_d[:])
            nc.sync.dma_start(mask[:], mask_d[:])
            for p in range(QPK):
                nc.sync.dma_start(qaug[D:D + 1, p, :], qaug_d[p:p + 1, :])
            nc.vector.memset(kaug[D:D + 1, :], 1.0)
            nc.vector.memset(vaug[:, :, D:D + 1], 1.0)

            # ---- K / V projections: kT/vT [64, L] ----
            for l in range(L // 512):
                sl = slice(512 * l, 512 * (l + 1))
                pk = ps.tile([128, 512], F32, tag="st", bufs=3)
                for kt in range(NKT):
                    nc.tensor.matmul(pk[:], wkv[:, kt, :], xt[:, kt, sl],
                                     start=(kt == 0), stop=(kt == NKT - 1))
                nc.vector.tensor_copy(kaug[0:D, sl], pk[0:D, :])
                nc.vector.tensor_copy(vtmp[:, sl], pk[D:128, :])
            # v natural layout via DMA transpose: vnat[:, jt, :] = vtmp[:, jt-block].T
            # (transpose dest must be 128B-aligned, so go via a packed tile)
            vnat = consts.tile([128, NJT, D], BF16)
            for jt in range(NJT):
                nc.sync.dma_start_transpose(vnat[:, jt, :],
                                            vtmp[:, 128 * jt:128 * (jt + 1)])
                nc.gpsimd.tensor_copy(vaug[:, jt, 0:D], vnat[:, jt, :])

            # ---- Q projection -> qaug rows 0..63 per head ----
            for m in range(2):
                for l in range(L // 512):
                    sl = slice(512 * l, 512 * (l + 1))
                    pq = ps.tile([128, 512], F32, tag="st", bufs=3)
                    for kt in range(NKT):
                        nc.tensor.matmul(pq[:], wq[:, kt, 128 * m:128 * (m + 1)],
                                         xt[:, kt, sl],
                                         start=(kt == 0), stop=(kt == NKT - 1))
                    nc.vector.tensor_copy(qaug[0:D, 2 * m, sl], pq[0:D, :])
                    nc.vector.tensor_copy(qaug[0:D, 2 * m + 1, sl], pq[D:128, :])

            # ---- attention per head / per 1024-wide i-chunk ----
            for p in range(QPK):
                for k2 in range(NCH):
                    i0 = BIG * k2
                    pv = ps.tile([D + 1, BIG], F32, tag="pv", bufs=1)
                    last_jt = 8 * k2 + 7
                    for jt in range(last_jt + 1):
                        off = max(0, 128 * jt - i0)
                        pieces = ([(off, 512), (512, BIG)] if off < 512
                                  else [(off, BIG)])
                        st = ps.tile([128, BIG], F32, tag="st", bufs=3)
                        for (a, b) in pieces:
                            nc.tensor.matmul(
                                st[:, a:b],
                                kaug[:, 128 * jt:128 * (jt + 1)],
                                qaug[:, p, i0 + a:i0 + b],
                                start=True, stop=True)
                        pt = pt_pool.tile([128, BIG], BF16)
                        nc.scalar.activation(
                            pt[:, off:BIG], st[:, off:BIG],
                            mybir.ActivationFunctionType.Exp,
                            bias=alibi[:, p * NJT + jt:p * NJT + jt + 1])
                        if 128 * jt >= i0:  # diagonal tile: causal mask
                            nc.vector.tensor_mul(pt[:, off:off + 128],
                                                 pt[:, off:off + 128], mask[:])
                        for (a, b) in pieces:
                            bank_last = (last_jt if b == BIG
                                         else min(8 * k2 + 3, last_jt))
                            nc.tensor.matmul(
                                pv[:, a:b], vaug[:, jt, :], pt[:, a:b],
                                start=(jt == 0), stop=(jt == bank_last))
                    # normalization: attnT[head rows, chunk] = pv[0:64]/pv[64]
                    pvs = norm_pool.tile([D, BIG], F32, tag="pvs")
                    nc.vector.tensor_copy(pvs[:], pv[0:D, :])
                    den = norm_pool.tile([1, BIG], F32, tag="den")
                    nc.vector.tensor_copy(den[:], pv[D:D + 1, :])
                    rec = norm_pool.tile([1, BIG], F32, tag="rec")
                    nc.vector.reciprocal_approx_fast(rec[:], den[:])
                    recb = norm_pool.tile([D, BIG], F32, tag="recb")
                    nc.gpsimd.partition_broadcast(recb[:], rec[:])
                    nc.vector.tensor_mul(
                        attnT[64 * (p % 2):64 * (p % 2) + D, p // 2,
                              i0:i0 + BIG],
                        pvs[:], recb[:])

            # ---- partial out-projection: yT[o, i] ----
            for m in range(H // 128):
                for l in range(L // 512):
                    sl = slice(512 * l, 512 * (l + 1))
                    py = ps.tile([128, 512], F32, tag="st", bufs=3)
                    for c2 in range(2):
                        nc.tensor.matmul(py[:], wo[:, c2, 128 * m:128 * (m + 1)],
                                         attnT[:, c2, sl],
                                         start=(c2 == 0), stop=(c2 == 1))
                    ys = y_pool.tile([128, 512], F32)
                    if (m * 4 + l) % 4 == 0:
                        nc.scalar.copy(ys[:], py[:])
                    else:
                        nc.vector.tensor_copy(ys[:], py[:])
                    nc.sync.dma_start(yt_d[128 * m:128 * (m + 1), sl], ys[:])

    nc.compile()
    return nc


_NC_CACHE = None


def _get_nc():
    global _NC_CACHE
    if _NC_CACHE is None:
        _NC_CACHE = build_bass()
    return _NC_CACHE


def make_in_maps(x, Wq, Wkv, Wo):
    x = np.asarray(x, np.float32)
    Wq = np.asarray(Wq, np.float32)
    Wkv = np.asarray(Wkv, np.float32)
    Wo = np.asarray(Wo, np.float32)

    slopes = 2.0 ** (-8.0 / N_HEAD * (np.arange(N_HEAD, dtype=np.float64) + 1.0))
    jpos = np.arange(128, dtype=np.float64)
    ipos = np.arange(L, dtype=np.float64)
    mask = np.where(jpos[:, None] <= jpos[None, :], 1.0, 0.0).astype(nbf16)

    in_maps = []
    for core in range(N_CORES):
        b, g = divmod(core, N_KV)
        heads = [N_KV * 0 + 4 * g + p for p in range(QPK)]  # 4g..4g+3
        xt = np.ascontiguousarray(x[b].T).astype(nbf16)
        wq = np.ascontiguousarray(
            (Wq[256 * g:256 * (g + 1), :] * SCALE).T).astype(nbf16)
        wkv = np.ascontiguousarray(Wkv[128 * g:128 * (g + 1), :].T).astype(nbf16)
        wo = np.ascontiguousarray(Wo[:, 256 * g:256 * (g + 1)].T).astype(nbf16)
        alibi = np.empty((128, QPK * NJT), np.float32)
        for p in range(QPK):
            s = slopes[heads[p]]
            for jt in range(NJT):
                alibi[:, p * NJT + jt] = (s * (128 * jt + jpos)).astype(np.float32)
        qaug = np.empty((QPK, L), nbf16)
        for p in range(QPK):
            s = slopes[heads[p]]
            qaug[p] = (-s * ipos - C_STAB).astype(nbf16)
        in_maps.append({
            "xt": xt, "wq": wq, "wkv": wkv, "wo": wo,
            "alibi": alibi, "qaug": qaug, "mask": mask,
        })
    return in_maps


def kernel(x, Wq, Wkv, Wo, _trace=False):
    _ensure_ntff_hook()
    nc = _get_nc()
    in_maps = make_in_maps(x, Wq, Wkv, Wo)
    res = run_bass_kernel_spmd(nc, in_maps, core_ids=list(range(N_CORES)),
                               trace=_trace)
    outs = [r["yt"] for r in res.results]  # each [H, L] = partial y.T
    y = np.empty((B, L, H), np.float32)
    for b in range(B):
        acc = outs[N_KV * b]
        for g in range(1, N_KV):
            acc = acc + outs[N_KV * b + g]
        y[b] = acc.T
    if _trace:
        kernel._last_result = res
    return y
